# revision 26
# baseline (speedup 1.0000x reference)
"""Trainium2 Bass kernel for nn_Compressor (sparse_attention, hierarchical window MLP).

Reference computation (per batch b, head h):
  windows w=0..510 over k[b,h] (S=8192, D=128), window length 32, stride 16
  x[w, l, :] = k[16w+l, :] + pe[l, :]
  5 stages of pairwise-merge MLP: x <- silu(x.reshape(-1, 256) @ w_down[i].T)
  out[w+1] = x @ w_stop.T   ; out[0] = 0 (prepended zero window)

Sharding: head-parallel across 8 cores (B*H = 32 -> 4 heads/core), weights
replicated, no cross-device comms.

Algebraic optimization (stage 0): adjacent row pairs (2t, 2t+1) are shared by
exactly two windows, always in the same even/odd role, so the linear part
  Z[:, t] = W0_even @ kT[:, 2t] + W0_odd @ kT[:, 2t+1]
is computed once per pair; the window-position part enters only through the
pe-bias folded into the ScalarE activation instruction:
  s0[:, (w, j)] = silu(Z[:, 8w+j] + (W0 @ pe_pair_j))

ScalarE is the bottleneck engine (1 elem/cycle/lane @ 1.2 GHz, no accel
modes): 31 silu planes x 511 windows x 4 heads = 63.4k elems/partition/core
(~53 us floor).  The kernel keeps ACT maximally dense:

  * all 4 heads are batched into every activation (FD = 4*511 = 2044,
    amortizing the ~300-cycle per-instruction bubble); plane 0 runs per-head
    so ACT starts as soon as the first head's k chunk lands,
  * stages run sequentially: per stage-0 iteration the PE work (8 matmuls)
    fits under the two activations even when the HAM clock-gate has the PE
    throttled to 1.2 GHz, so stage 0 is ACT-bound at any clock; stages 1-4
    keep the PE dense enough that the clock stays at 2.4 GHz,
  * absorber weight-loads observe each k-chunk's DMA semaphore before the
    Z matmuls, so the matmuls themselves carry no waits and issue
    back-to-back (a fused wait blocks LDWEIGHTS pull-ahead, costing ~170ns
    per matmul),
  * PSUM rotates as two 4-bank slots (one bank per head); slot rotation is
    ordered so a consumer only WAR-waits on work that drained long before,
  * k arrives pre-transposed from the host ([d, l, w] planes) so input DMA
    is plain contiguous 2KB-per-partition chunks; weights ride the same
    queue first so nothing gates the first matmul,
  * outputs leave partition-major ([p, q, o], un-permuted on the host for
    free) so each head is one 128x2KB-descriptor DMA issued right after its
    PSUM->SBUF copy - the drain tail is a few us instead of ~20.
"""

import numpy as np

B, H, S, D = 2, 16, 8192, 128
BH = B * H
NCORES = 8
HPC = BH // NCORES  # heads per core = 4
NB = (S - 32) // 16 + 1  # 511 sliding windows
NW = NB + 1  # 512 output rows per head (incl. zero window)

# w_stop output chunking: window ranges per PE (stationary) chunk
QRANGES = [(0, 128), (128, 128), (256, 128), (384, 127)]

_BASS_CACHE = {}


def _build_bass():
    import concourse.bacc as bacc
    import concourse.mybir as mybir
    import concourse.tile as tile

    f32 = mybir.dt.float32
    bf16 = mybir.dt.bfloat16
    SILU = mybir.ActivationFunctionType.Silu

    nc = bacc.Bacc()
    # ktt[hh, d, l, w] = bf16 k[head, 16w + l, d]  (host pre-transposed)
    ktt = nc.dram_tensor("ktt", [HPC, 128, 16, 512], bf16, kind="ExternalInput")
    # wdt[k, i, half, o] = w_down[i][o, 128*half + k]
    wdt = nc.dram_tensor("wdt", [128, 5, 2, 128], bf16, kind="ExternalInput")
    pe0 = nc.dram_tensor("pe0", [128, 16], f32, kind="ExternalInput")
    wst = nc.dram_tensor("wst", [128, 128], bf16, kind="ExternalInput")
    # partition-major outputs: ob[hh][p, q, o] = out_row(128q + p), host
    # un-permutes; this keeps the output DMA at 2KB descriptors
    oqs = [
        nc.dram_tensor(f"o{hh}", [128, 4, 128], f32, kind="ExternalOutput")
        for hh in range(HPC)
    ]

    with tile.TileContext(nc) as tc:
        with (
            tc.tile_pool(name="consts", bufs=1) as consts,
            tc.tile_pool(name="ktp", bufs=1) as ktp,
            tc.tile_pool(name="s0p", bufs=1) as s0p,
            tc.tile_pool(name="stp", bufs=1) as stp,
            tc.tile_pool(name="outp", bufs=2) as outp,
            tc.tile_pool(name="zps", bufs=2, space="PSUM") as zps,
        ):
            # the first matmul needs only the stage-0 weights + the first k
            # chunk: both ride the fast sync queue, stage-0 weights first;
            # the rest of the weights follow on the slower scalar/gpsimd
            # dynamic queues (needed tens of us later)
            wd0_sb = consts.tile([128, 2, 128], bf16, name="wd0_sb")
            nc.sync.dma_start(out=wd0_sb, in_=wdt[:, 0, :, :])
            pe0_sb = consts.tile([128, 16], f32, name="pe0_sb")
            nc.sync.dma_start(out=pe0_sb, in_=pe0[:])
            wdr_sb = consts.tile([128, 4, 2, 128], bf16, name="wdr_sb")
            nc.gpsimd.dma_start(out=wdr_sb, in_=wdt[:, 1:5, :, :])
            wst_sb = consts.tile([128, 128], bf16, name="wst_sb")
            nc.gpsimd.dma_start(out=wst_sb, in_=wst[:])

            def wd(i, half):
                return wd0_sb[:, half, :] if i == 0 else wdr_sb[:, i - 1, half, :]

            # HAM prewarm: dummy weight loads keep the PE array active from
            # right after the preamble
            dw = consts.tile([128, 128], bf16, name="dw")
            nc.vector.memset(dw, 0.0)
            for _ in range(14):
                nc.tensor.ldweights(weights=dw)

            def filler(anchor, n):
                # dummy weight loads anchored on the PREVIOUS iteration's
                # activation output: their wait is exactly the WAR gate the
                # next iteration's matmuls sit on anyway, so they never delay
                # real work - they just convert that forced idle into PE
                # activity so the HAM clock-gate keeps the array at 2.4 GHz
                for _ in range(n):
                    nc.tensor.ldweights(weights=anchor)

            # full k for all 4 heads, l-planar: ktf[d, hh, l, w]; all chunks
            # on the sync queue (the only fast DMA ring - scalar/gpsimd
            # dynamic queues add ~3us of latency)
            ktf = ktp.tile([128, HPC, 16, 512], bf16, name="ktf")
            for e in range(8):
                for hh in range(HPC):
                    nc.sync.dma_start(
                        out=ktf[:, hh, 2 * e : 2 * e + 2, :],
                        in_=ktt[hh, :, 2 * e : 2 * e + 2, :],
                    )

            # stage outputs, layout [d, plane, head, w]
            s0 = s0p.tile([128, 16, HPC, NB], bf16, name="s0")
            sts = [s0]
            for st in range(1, 5):
                sts.append(
                    stp.tile([128, 16 >> st, HPC, NB], bf16, name=f"s{st}")
                )

            # ---- stage 0, e = 0: per-head ramp (ACT starts ~4us earlier) ----
            zp0 = zps.tile([128, HPC, 512], f32, name="zp0", tag="zp")
            for hh in range(HPC):
                # absorber: observe this chunk's DMA semaphore on PE
                nc.tensor.ldweights(weights=ktf[:, hh, 0, 0:128])
                for half in range(2):
                    nc.tensor.matmul(
                        zp0[:, hh, :],
                        lhsT=wd(0, half),
                        rhs=ktf[:, hh, half, :],
                        start=(half == 0),
                        stop=(half == 1),
                    )
                nc.scalar.activation(
                    out=s0[:, 0, hh, :], in_=zp0[:, hh, 0:NB], func=SILU,
                    bias=pe0_sb[:, 0:1], scale=1.0,
                )
                nc.scalar.activation(
                    out=s0[:, 8, hh, :], in_=zp0[:, hh, 1 : NB + 1], func=SILU,
                    bias=pe0_sb[:, 8:9], scale=1.0,
                )

            def stage0_iter(e, split=False):
                zp = zps.tile([128, HPC, 512], f32, name=f"zp{e}", tag="zp")
                if split:
                    # head-major matmuls: a head-pair's activations fire as
                    # soon as that pair's chunks have landed (supply-paced
                    # ramp region)
                    for hh in range(HPC):
                        nc.tensor.ldweights(weights=ktf[:, hh, 2 * e, 0:128])
                        for half in range(2):
                            nc.tensor.matmul(
                                zp[:, hh, :],
                                lhsT=wd(0, half),
                                rhs=ktf[:, hh, 2 * e + half, :],
                                start=(half == 0),
                                stop=(half == 1),
                            )
                else:
                    for hh in range(HPC):
                        nc.tensor.ldweights(weights=ktf[:, hh, 2 * e, 0:128])
                    for half in range(2):
                        for hh in range(HPC):
                            nc.tensor.matmul(
                                zp[:, hh, :],
                                lhsT=wd(0, half),
                                rhs=ktf[:, hh, 2 * e + half, :],
                                start=(half == 0),
                                stop=(half == 1),
                            )
                groups = [(0, 2), (2, 4)] if split else [(0, 4)]
                for a, b in groups:
                    nc.scalar.activation(
                        out=s0[:, e, a:b, :], in_=zp[:, a:b, 0:NB], func=SILU,
                        bias=pe0_sb[:, e : e + 1], scale=1.0,
                    )
                    nc.scalar.activation(
                        out=s0[:, e + 8, a:b, :], in_=zp[:, a:b, 1 : NB + 1],
                        func=SILU,
                        bias=pe0_sb[:, e + 8 : e + 9], scale=1.0,
                    )
                filler(s0[:, e - 1, 0, 0:128], 5)
                filler(s0[:, e + 7, 0, 0:128], 5)

            def merge_iter(st, p, prev_anchor=None, split=False):
                prev, cur = sts[st - 1], sts[st]
                ps = zps.tile([128, HPC, 512], f32, name=f"ps{st}_{p}", tag="zp")
                for half in range(2):
                    for hh in range(HPC):
                        nc.tensor.matmul(
                            ps[:, hh, 0:NB],
                            lhsT=wd(st, half),
                            rhs=prev[:, 2 * p + half, hh, :],
                            start=(half == 0),
                            stop=(half == 1),
                        )
                if split:
                    # head-pair halves so downstream per-head work starts
                    # ~1us earlier (used for the last stage-3 plane)
                    nc.scalar.activation(
                        out=cur[:, p, 0:2, :], in_=ps[:, 0:2, 0:NB], func=SILU,
                    )
                    nc.scalar.activation(
                        out=cur[:, p, 2:4, :], in_=ps[:, 2:4, 0:NB], func=SILU,
                    )
                else:
                    nc.scalar.activation(
                        out=cur[:, p, :, :], in_=ps[:, :, 0:NB], func=SILU,
                    )
                if prev_anchor is not None:
                    filler(prev_anchor, 4)

            # ---- stage 0 e>=1 (sequential: stage 0 is ACT-bound per
            # iteration even with the PE clock-gated cold, so no stage-1
            # work needs to be woven in) ----
            for e in range(1, 8):
                stage0_iter(e)

            # ---- stages 1..3 ----
            anchor = s0[:, 15, 0, 0:128]
            for st in range(1, 4):
                cur = sts[st]
                for p in range(16 >> st):
                    merge_iter(st, p, prev_anchor=anchor,
                               split=(st == 3 and p == 1))
                    anchor = cur[:, p, 0, 0:128]

            # ---- stage 4 + w_stop + output, pipelined per head ----
            s3, s4 = sts[3], sts[4]
            ps4 = []
            for hh in range(HPC):
                if hh >= 2:
                    # slot WAR: ps4[hh] reuses ps4[hh-2]'s slot; its reader
                    # (act4 hh-2) must be emitted before the new allocation
                    nc.scalar.activation(
                        out=s4[:, 0, hh - 2, :], in_=ps4[hh - 2][:, 0:NB],
                        func=SILU,
                    )
                p4 = zps.tile([128, 512], f32, name=f"ps4_{hh}", tag="zp")
                ps4.append(p4)
                for half in range(2):
                    nc.tensor.matmul(
                        p4[:, 0:NB],
                        lhsT=wd(4, half),
                        rhs=s3[:, half, hh, :],
                        start=(half == 0),
                        stop=(half == 1),
                    )
            for hh in range(2, HPC):
                nc.scalar.activation(
                    out=s4[:, 0, hh, :], in_=ps4[hh][:, 0:NB], func=SILU,
                )

            for hh in range(HPC):
                ps2 = zps.tile([128, 4, 128], f32, name=f"ps2_{hh}", tag="zp")
                for q, (w0, wq) in enumerate(QRANGES):
                    nc.tensor.matmul(
                        ps2[:wq, q, :],
                        lhsT=s4[:, 0, hh, w0 : w0 + wq],
                        rhs=wst_sb,
                        start=True,
                        stop=True,
                    )
                outsb = outp.tile([128, 4, 128], f32, name=f"outsb{hh}", tag="ob")
                nc.vector.tensor_copy(out=outsb, in_=ps2)
                filler(s4[:, 0, hh, 0:128], 3)
                nc.sync.dma_start(out=oqs[hh][:], in_=outsb)

    if not nc.is_finalized():
        nc.finalize()
    return nc


def _prep_host_inputs(k, pe, w_down, w_stop):
    import ml_dtypes

    bf16 = ml_dtypes.bfloat16
    k = np.asarray(k, dtype=np.float32)
    pe = np.asarray(pe, dtype=np.float32)
    w_down = np.asarray(w_down, dtype=np.float32)
    w_stop = np.asarray(w_stop, dtype=np.float32)

    # ktt[head, d, l, w] = k[head, 16w + l, d], cast bf16 (RNE): device DMA
    # is then a plain contiguous copy per (plane, head) chunk
    ktt = np.ascontiguousarray(
        k.reshape(BH, 512, 16, D).transpose(0, 3, 2, 1)
    ).astype(bf16)
    # wdt[kk, i, h, o] = w_down[i][o, 128h + kk]
    wdt = np.ascontiguousarray(
        w_down.reshape(5, 128, 2, 128).transpose(3, 0, 2, 1)
    ).astype(bf16)
    # pe0[o, j] = sum_i w_down[0][o, i] * concat(pe[2j], pe[2j+1])[i]
    pe_pairs = pe.reshape(16, 256).astype(np.float64)
    pe0 = (w_down[0].astype(np.float64) @ pe_pairs.T).astype(np.float32)
    wst = np.ascontiguousarray(w_stop.T).astype(bf16)
    return ktt, wdt, pe0, wst


def run(k, pe, w_down, w_stop, trace=False, trace_kwargs=None):
    from concourse.bass_utils import run_bass_kernel_spmd

    ktt, wdt, pe0, wst = _prep_host_inputs(k, pe, w_down, w_stop)

    if "nc" not in _BASS_CACHE:
        _BASS_CACHE["nc"] = _build_bass()
    nc = _BASS_CACHE["nc"]

    in_maps = [
        {
            "ktt": np.ascontiguousarray(ktt[HPC * c : HPC * (c + 1)]),
            "wdt": wdt,
            "pe0": pe0,
            "wst": wst,
        }
        for c in range(NCORES)
    ]
    res = run_bass_kernel_spmd(
        nc, in_maps, core_ids=list(range(NCORES)), trace=trace,
        **(trace_kwargs or {}),
    )
    out = np.empty((BH, NW, D), dtype=np.float32)
    for c in range(NCORES):
        r = res.results[c]
        for hh in range(HPC):
            row = HPC * c + hh
            out[row, 0, :] = 0.0
            # ob[p, q, o] -> out rows 1 + 128q + p  (row 512 doesn't exist)
            ob = r[f"o{hh}"].transpose(1, 0, 2).reshape(512, D)
            out[row, 1:, :] = ob[:NB]
    out = out.reshape(B, H, NW, D)
    return out, res


def kernel(k, pe, w_down, w_stop):
    out, _ = run(k, pe, w_down, w_stop, trace=False)
    return out


# revision 27
# speedup vs baseline: 1.0000x; 1.0000x over previous
"""Trainium2 Bass kernel for nn_Compressor (sparse_attention, hierarchical window MLP).

Reference computation (per batch b, head h):
  windows w=0..510 over k[b,h] (S=8192, D=128), window length 32, stride 16
  x[w, l, :] = k[16w+l, :] + pe[l, :]
  5 stages of pairwise-merge MLP: x <- silu(x.reshape(-1, 256) @ w_down[i].T)
  out[w+1] = x @ w_stop.T   ; out[0] = 0 (prepended zero window)

Sharding: head-parallel across 8 cores (B*H = 32 -> 4 heads/core), weights
replicated, no cross-device comms.

Algebraic optimization (stage 0): adjacent row pairs (2t, 2t+1) are shared by
exactly two windows, always in the same even/odd role, so the linear part
  Z[:, t] = W0_even @ kT[:, 2t] + W0_odd @ kT[:, 2t+1]
is computed once per pair; the window-position part enters only through the
pe-bias folded into the ScalarE activation instruction:
  s0[:, (w, j)] = silu(Z[:, 8w+j] + (W0 @ pe_pair_j))

ScalarE is the bottleneck engine (1 elem/cycle/lane @ 1.2 GHz, no accel
modes): 31 silu planes x 511 windows x 4 heads = 63.4k elems/partition/core
(~53 us floor).  The kernel keeps ACT maximally dense:

  * all 4 heads are batched into every activation (FD = 4*511 = 2044,
    amortizing the ~300-cycle per-instruction bubble); plane 0 runs per-head
    so ACT starts as soon as the first head's k chunk lands,
  * stages run sequentially: per stage-0 iteration the PE work (8 matmuls)
    fits under the two activations even when the HAM clock-gate has the PE
    throttled to 1.2 GHz, so stage 0 is ACT-bound at any clock; stages 1-4
    keep the PE dense enough that the clock stays at 2.4 GHz,
  * absorber weight-loads observe each k-chunk's DMA semaphore before the
    Z matmuls, so the matmuls themselves carry no waits and issue
    back-to-back (a fused wait blocks LDWEIGHTS pull-ahead, costing ~170ns
    per matmul),
  * PSUM rotates as two 4-bank slots (one bank per head); slot rotation is
    ordered so a consumer only WAR-waits on work that drained long before,
  * k arrives pre-transposed from the host ([d, l, w] planes) so input DMA
    is plain contiguous 2KB-per-partition chunks; weights ride the same
    queue first so nothing gates the first matmul,
  * outputs leave partition-major ([p, q, o], un-permuted on the host for
    free) so each head is one 128x2KB-descriptor DMA issued right after its
    PSUM->SBUF copy - the drain tail is a few us instead of ~20.
"""

import numpy as np

B, H, S, D = 2, 16, 8192, 128
BH = B * H
NCORES = 8
HPC = BH // NCORES  # heads per core = 4
NB = (S - 32) // 16 + 1  # 511 sliding windows
NW = NB + 1  # 512 output rows per head (incl. zero window)

# w_stop output chunking: window ranges per PE (stationary) chunk
QRANGES = [(0, 128), (128, 128), (256, 128), (384, 127)]

_BASS_CACHE = {}


def _build_bass():
    import concourse.bacc as bacc
    import concourse.mybir as mybir
    import concourse.tile as tile

    f32 = mybir.dt.float32
    bf16 = mybir.dt.bfloat16
    SILU = mybir.ActivationFunctionType.Silu

    nc = bacc.Bacc()
    # ktt[hh, d, l, w] = bf16 k[head, 16w + l, d]  (host pre-transposed)
    ktt = nc.dram_tensor("ktt", [HPC, 128, 16, 512], bf16, kind="ExternalInput")
    # wdt[k, i, half, o] = w_down[i][o, 128*half + k]
    wdt = nc.dram_tensor("wdt", [128, 5, 2, 128], bf16, kind="ExternalInput")
    pe0 = nc.dram_tensor("pe0", [128, 16], f32, kind="ExternalInput")
    wst = nc.dram_tensor("wst", [128, 128], bf16, kind="ExternalInput")
    # partition-major outputs: ob[hh][p, q, o] = out_row(128q + p), host
    # un-permutes; this keeps the output DMA at 2KB descriptors
    oqs = [
        nc.dram_tensor(f"o{hh}", [128, 4, 128], f32, kind="ExternalOutput")
        for hh in range(HPC)
    ]

    with tile.TileContext(nc) as tc:
        with (
            tc.tile_pool(name="consts", bufs=1) as consts,
            tc.tile_pool(name="ktp", bufs=1) as ktp,
            tc.tile_pool(name="s0p", bufs=1) as s0p,
            tc.tile_pool(name="stp", bufs=1) as stp,
            tc.tile_pool(name="outp", bufs=2) as outp,
            tc.tile_pool(name="zps", bufs=2, space="PSUM") as zps,
        ):
            # the first matmul needs only the stage-0 weights + the first k
            # chunk: both ride the fast sync queue, stage-0 weights first;
            # the rest of the weights follow on the slower scalar/gpsimd
            # dynamic queues (needed tens of us later)
            wd0_sb = consts.tile([128, 2, 128], bf16, name="wd0_sb")
            nc.sync.dma_start(out=wd0_sb, in_=wdt[:, 0, :, :])
            pe0_sb = consts.tile([128, 16], f32, name="pe0_sb")
            nc.sync.dma_start(out=pe0_sb, in_=pe0[:])
            wdr_sb = consts.tile([128, 4, 2, 128], bf16, name="wdr_sb")
            nc.scalar.dma_start(out=wdr_sb, in_=wdt[:, 1:5, :, :])
            wst_sb = consts.tile([128, 128], bf16, name="wst_sb")
            nc.gpsimd.dma_start(out=wst_sb, in_=wst[:])

            def wd(i, half):
                return wd0_sb[:, half, :] if i == 0 else wdr_sb[:, i - 1, half, :]

            # HAM prewarm: dummy weight loads keep the PE array active from
            # right after the preamble
            dw = consts.tile([128, 128], bf16, name="dw")
            nc.vector.memset(dw, 0.0)
            for _ in range(14):
                nc.tensor.ldweights(weights=dw)

            def filler(anchor, n):
                # dummy weight loads anchored on the PREVIOUS iteration's
                # activation output: their wait is exactly the WAR gate the
                # next iteration's matmuls sit on anyway, so they never delay
                # real work - they just convert that forced idle into PE
                # activity so the HAM clock-gate keeps the array at 2.4 GHz
                for _ in range(n):
                    nc.tensor.ldweights(weights=anchor)

            # full k for all 4 heads, l-planar: ktf[d, hh, l, w]; all chunks
            # on the sync queue (the only fast DMA ring - scalar/gpsimd
            # dynamic queues add ~3us of latency)
            ktf = ktp.tile([128, HPC, 16, 512], bf16, name="ktf")
            for e in range(8):
                for hh in range(HPC):
                    nc.sync.dma_start(
                        out=ktf[:, hh, 2 * e : 2 * e + 2, :],
                        in_=ktt[hh, :, 2 * e : 2 * e + 2, :],
                    )

            # stage outputs, layout [d, plane, head, w]
            s0 = s0p.tile([128, 16, HPC, NB], bf16, name="s0")
            sts = [s0]
            for st in range(1, 5):
                sts.append(
                    stp.tile([128, 16 >> st, HPC, NB], bf16, name=f"s{st}")
                )

            # ---- stage 0, e = 0: per-head ramp (ACT starts ~4us earlier) ----
            zp0 = zps.tile([128, HPC, 512], f32, name="zp0", tag="zp")
            for hh in range(HPC):
                # absorber: observe this chunk's DMA semaphore on PE
                nc.tensor.ldweights(weights=ktf[:, hh, 0, 0:128])
                for half in range(2):
                    nc.tensor.matmul(
                        zp0[:, hh, :],
                        lhsT=wd(0, half),
                        rhs=ktf[:, hh, half, :],
                        start=(half == 0),
                        stop=(half == 1),
                    )
                nc.scalar.activation(
                    out=s0[:, 0, hh, :], in_=zp0[:, hh, 0:NB], func=SILU,
                    bias=pe0_sb[:, 0:1], scale=1.0,
                )
                nc.scalar.activation(
                    out=s0[:, 8, hh, :], in_=zp0[:, hh, 1 : NB + 1], func=SILU,
                    bias=pe0_sb[:, 8:9], scale=1.0,
                )

            def stage0_iter(e, split=False):
                zp = zps.tile([128, HPC, 512], f32, name=f"zp{e}", tag="zp")
                if split:
                    # head-major matmuls: a head-pair's activations fire as
                    # soon as that pair's chunks have landed (supply-paced
                    # ramp region)
                    for hh in range(HPC):
                        nc.tensor.ldweights(weights=ktf[:, hh, 2 * e, 0:128])
                        for half in range(2):
                            nc.tensor.matmul(
                                zp[:, hh, :],
                                lhsT=wd(0, half),
                                rhs=ktf[:, hh, 2 * e + half, :],
                                start=(half == 0),
                                stop=(half == 1),
                            )
                else:
                    for hh in range(HPC):
                        nc.tensor.ldweights(weights=ktf[:, hh, 2 * e, 0:128])
                    for half in range(2):
                        for hh in range(HPC):
                            nc.tensor.matmul(
                                zp[:, hh, :],
                                lhsT=wd(0, half),
                                rhs=ktf[:, hh, 2 * e + half, :],
                                start=(half == 0),
                                stop=(half == 1),
                            )
                groups = [(0, 2), (2, 4)] if split else [(0, 4)]
                for a, b in groups:
                    nc.scalar.activation(
                        out=s0[:, e, a:b, :], in_=zp[:, a:b, 0:NB], func=SILU,
                        bias=pe0_sb[:, e : e + 1], scale=1.0,
                    )
                    nc.scalar.activation(
                        out=s0[:, e + 8, a:b, :], in_=zp[:, a:b, 1 : NB + 1],
                        func=SILU,
                        bias=pe0_sb[:, e + 8 : e + 9], scale=1.0,
                    )
                filler(s0[:, e - 1, 0, 0:128], 5)
                filler(s0[:, e + 7, 0, 0:128], 5)

            def merge_iter(st, p, prev_anchor=None, split=False):
                prev, cur = sts[st - 1], sts[st]
                ps = zps.tile([128, HPC, 512], f32, name=f"ps{st}_{p}", tag="zp")
                for half in range(2):
                    for hh in range(HPC):
                        nc.tensor.matmul(
                            ps[:, hh, 0:NB],
                            lhsT=wd(st, half),
                            rhs=prev[:, 2 * p + half, hh, :],
                            start=(half == 0),
                            stop=(half == 1),
                        )
                if split:
                    # head-pair halves so downstream per-head work starts
                    # ~1us earlier (used for the last stage-3 plane)
                    nc.scalar.activation(
                        out=cur[:, p, 0:2, :], in_=ps[:, 0:2, 0:NB], func=SILU,
                    )
                    nc.scalar.activation(
                        out=cur[:, p, 2:4, :], in_=ps[:, 2:4, 0:NB], func=SILU,
                    )
                else:
                    nc.scalar.activation(
                        out=cur[:, p, :, :], in_=ps[:, :, 0:NB], func=SILU,
                    )
                if prev_anchor is not None:
                    filler(prev_anchor, 4)

            # ---- stage 0 e>=1 (sequential: stage 0 is ACT-bound per
            # iteration even with the PE clock-gated cold, so no stage-1
            # work needs to be woven in) ----
            for e in range(1, 8):
                stage0_iter(e)

            # ---- stages 1..3 ----
            anchor = s0[:, 15, 0, 0:128]
            for st in range(1, 4):
                cur = sts[st]
                for p in range(16 >> st):
                    merge_iter(st, p, prev_anchor=anchor)
                    anchor = cur[:, p, 0, 0:128]

            # ---- stage 4 + w_stop + output, pipelined per head ----
            s3, s4 = sts[3], sts[4]
            ps4 = []
            for hh in range(HPC):
                if hh >= 2:
                    # slot WAR: ps4[hh] reuses ps4[hh-2]'s slot; its reader
                    # (act4 hh-2) must be emitted before the new allocation
                    nc.scalar.activation(
                        out=s4[:, 0, hh - 2, :], in_=ps4[hh - 2][:, 0:NB],
                        func=SILU,
                    )
                p4 = zps.tile([128, 512], f32, name=f"ps4_{hh}", tag="zp")
                ps4.append(p4)
                for half in range(2):
                    nc.tensor.matmul(
                        p4[:, 0:NB],
                        lhsT=wd(4, half),
                        rhs=s3[:, half, hh, :],
                        start=(half == 0),
                        stop=(half == 1),
                    )
            for hh in range(2, HPC):
                nc.scalar.activation(
                    out=s4[:, 0, hh, :], in_=ps4[hh][:, 0:NB], func=SILU,
                )

            for hh in range(HPC):
                ps2 = zps.tile([128, 4, 128], f32, name=f"ps2_{hh}", tag="zp")
                for q, (w0, wq) in enumerate(QRANGES):
                    nc.tensor.matmul(
                        ps2[:wq, q, :],
                        lhsT=s4[:, 0, hh, w0 : w0 + wq],
                        rhs=wst_sb,
                        start=True,
                        stop=True,
                    )
                outsb = outp.tile([128, 4, 128], f32, name=f"outsb{hh}", tag="ob")
                nc.vector.tensor_copy(out=outsb, in_=ps2)
                filler(s4[:, 0, hh, 0:128], 3)
                nc.sync.dma_start(out=oqs[hh][:], in_=outsb)

    if not nc.is_finalized():
        nc.finalize()
    return nc


def _prep_host_inputs(k, pe, w_down, w_stop):
    import ml_dtypes

    bf16 = ml_dtypes.bfloat16
    k = np.asarray(k, dtype=np.float32)
    pe = np.asarray(pe, dtype=np.float32)
    w_down = np.asarray(w_down, dtype=np.float32)
    w_stop = np.asarray(w_stop, dtype=np.float32)

    # ktt[head, d, l, w] = k[head, 16w + l, d], cast bf16 (RNE): device DMA
    # is then a plain contiguous copy per (plane, head) chunk
    ktt = np.ascontiguousarray(
        k.reshape(BH, 512, 16, D).transpose(0, 3, 2, 1)
    ).astype(bf16)
    # wdt[kk, i, h, o] = w_down[i][o, 128h + kk]
    wdt = np.ascontiguousarray(
        w_down.reshape(5, 128, 2, 128).transpose(3, 0, 2, 1)
    ).astype(bf16)
    # pe0[o, j] = sum_i w_down[0][o, i] * concat(pe[2j], pe[2j+1])[i]
    pe_pairs = pe.reshape(16, 256).astype(np.float64)
    pe0 = (w_down[0].astype(np.float64) @ pe_pairs.T).astype(np.float32)
    wst = np.ascontiguousarray(w_stop.T).astype(bf16)
    return ktt, wdt, pe0, wst


def run(k, pe, w_down, w_stop, trace=False, trace_kwargs=None):
    from concourse.bass_utils import run_bass_kernel_spmd

    ktt, wdt, pe0, wst = _prep_host_inputs(k, pe, w_down, w_stop)

    if "nc" not in _BASS_CACHE:
        _BASS_CACHE["nc"] = _build_bass()
    nc = _BASS_CACHE["nc"]

    in_maps = [
        {
            "ktt": np.ascontiguousarray(ktt[HPC * c : HPC * (c + 1)]),
            "wdt": wdt,
            "pe0": pe0,
            "wst": wst,
        }
        for c in range(NCORES)
    ]
    res = run_bass_kernel_spmd(
        nc, in_maps, core_ids=list(range(NCORES)), trace=trace,
        **(trace_kwargs or {}),
    )
    out = np.empty((BH, NW, D), dtype=np.float32)
    for c in range(NCORES):
        r = res.results[c]
        for hh in range(HPC):
            row = HPC * c + hh
            out[row, 0, :] = 0.0
            # ob[p, q, o] -> out rows 1 + 128q + p  (row 512 doesn't exist)
            ob = r[f"o{hh}"].transpose(1, 0, 2).reshape(512, D)
            out[row, 1:, :] = ob[:NB]
    out = out.reshape(B, H, NW, D)
    return out, res


def kernel(k, pe, w_down, w_stop):
    out, _ = run(k, pe, w_down, w_stop, trace=False)
    return out


# revision 29
# speedup vs baseline: 1.0317x; 1.0316x over previous
"""Trainium2 Bass kernel for nn_Compressor (sparse_attention, hierarchical window MLP).

Reference computation (per batch b, head h):
  windows w=0..510 over k[b,h] (S=8192, D=128), window length 32, stride 16
  x[w, l, :] = k[16w+l, :] + pe[l, :]
  5 stages of pairwise-merge MLP: x <- silu(x.reshape(-1, 256) @ w_down[i].T)
  out[w+1] = x @ w_stop.T   ; out[0] = 0 (prepended zero window)

Sharding: head-parallel across 8 cores (B*H = 32 -> 4 heads/core), weights
replicated, no cross-device comms.

Algebraic optimization (stage 0): adjacent row pairs (2t, 2t+1) are shared by
exactly two windows, always in the same even/odd role, so the linear part
  Z[:, t] = W0_even @ kT[:, 2t] + W0_odd @ kT[:, 2t+1]
is computed once per pair; the window-position part enters only through the
pe-bias folded into the ScalarE activation instruction:
  s0[:, (w, j)] = silu(Z[:, 8w+j] + (W0 @ pe_pair_j))

ScalarE is the bottleneck engine (1 elem/cycle/lane @ 1.2 GHz, no accel
modes): 31 silu planes x 511 windows x 4 heads = 63.4k elems/partition/core
(~53 us floor).  The kernel keeps ACT maximally dense:

  * all 4 heads are batched into every activation (FD = 4*511 = 2044,
    amortizing the ~300-cycle per-instruction bubble); plane 0 runs per-head
    so ACT starts as soon as the first head's k chunk lands,
  * stages run sequentially: per stage-0 iteration the PE work (8 matmuls)
    fits under the two activations even when the HAM clock-gate has the PE
    throttled to 1.2 GHz, so stage 0 is ACT-bound at any clock; stages 1-4
    keep the PE dense enough that the clock stays at 2.4 GHz,
  * absorber weight-loads observe each k-chunk's DMA semaphore before the
    Z matmuls, so the matmuls themselves carry no waits and issue
    back-to-back (a fused wait blocks LDWEIGHTS pull-ahead, costing ~170ns
    per matmul),
  * PSUM rotates as two 4-bank slots (one bank per head); slot rotation is
    ordered so a consumer only WAR-waits on work that drained long before,
  * k arrives pre-transposed from the host ([d, l, w] planes) so input DMA
    is plain contiguous 2KB-per-partition chunks; weights ride the same
    queue first so nothing gates the first matmul,
  * outputs leave partition-major ([p, q, o], un-permuted on the host for
    free) so each head is one 128x2KB-descriptor DMA issued right after its
    PSUM->SBUF copy - the drain tail is a few us instead of ~20.
"""

import numpy as np

B, H, S, D = 2, 16, 8192, 128
BH = B * H
NCORES = 8
HPC = BH // NCORES  # heads per core = 4
NB = (S - 32) // 16 + 1  # 511 sliding windows
NW = NB + 1  # 512 output rows per head (incl. zero window)

# w_stop output chunking: window ranges per PE (stationary) chunk
QRANGES = [(0, 128), (128, 128), (256, 128), (384, 127)]

_BASS_CACHE = {}


def _build_bass():
    import concourse.bacc as bacc
    import concourse.mybir as mybir
    import concourse.tile as tile

    f32 = mybir.dt.float32
    bf16 = mybir.dt.bfloat16
    SILU = mybir.ActivationFunctionType.Silu

    nc = bacc.Bacc()
    # ktt[hh, d, l, w] = bf16 k[head, 16w + l, d]  (host pre-transposed)
    ktt = nc.dram_tensor("ktt", [HPC, 128, 16, 512], bf16, kind="ExternalInput")
    # wdt[k, i, half, o] = w_down[i][o, 128*half + k]
    wdt = nc.dram_tensor("wdt", [128, 5, 2, 128], bf16, kind="ExternalInput")
    pe0 = nc.dram_tensor("pe0", [128, 16], f32, kind="ExternalInput")
    wst = nc.dram_tensor("wst", [128, 128], bf16, kind="ExternalInput")
    # partition-major outputs: ob[hh][p, q, o] = out_row(128q + p), host
    # un-permutes; this keeps the output DMA at 2KB descriptors
    oqs = [
        nc.dram_tensor(f"o{hh}", [128, 4, 128], f32, kind="ExternalOutput")
        for hh in range(HPC)
    ]

    with tile.TileContext(nc) as tc:
        with (
            tc.tile_pool(name="consts", bufs=1) as consts,
            tc.tile_pool(name="ktp", bufs=1) as ktp,
            tc.tile_pool(name="s0p", bufs=1) as s0p,
            tc.tile_pool(name="stp", bufs=1) as stp,
            tc.tile_pool(name="outp", bufs=2) as outp,
            tc.tile_pool(name="zps", bufs=2, space="PSUM") as zps,
        ):
            # the first matmul needs only the stage-0 weights + the first k
            # chunk: both ride the fast sync queue, stage-0 weights first;
            # the rest of the weights follow on the slower scalar/gpsimd
            # dynamic queues (needed tens of us later)
            wd0_sb = consts.tile([128, 2, 128], bf16, name="wd0_sb")
            nc.sync.dma_start(out=wd0_sb, in_=wdt[:, 0, :, :])
            pe0_sb = consts.tile([128, 16], f32, name="pe0_sb")
            nc.sync.dma_start(out=pe0_sb, in_=pe0[:])
            wdr_sb = consts.tile([128, 4, 2, 128], bf16, name="wdr_sb")
            nc.scalar.dma_start(out=wdr_sb, in_=wdt[:, 1:5, :, :])
            wst_sb = consts.tile([128, 128], bf16, name="wst_sb")
            nc.gpsimd.dma_start(out=wst_sb, in_=wst[:])

            def wd(i, half):
                return wd0_sb[:, half, :] if i == 0 else wdr_sb[:, i - 1, half, :]

            # HAM prewarm: dummy weight loads keep the PE array active from
            # right after the preamble
            dw = consts.tile([128, 128], bf16, name="dw")
            nc.vector.memset(dw, 0.0)
            for _ in range(14):
                nc.tensor.ldweights(weights=dw)

            def filler(anchor, n):
                # dummy weight loads anchored on the PREVIOUS iteration's
                # activation output: their wait is exactly the WAR gate the
                # next iteration's matmuls sit on anyway, so they never delay
                # real work - they just convert that forced idle into PE
                # activity so the HAM clock-gate keeps the array at 2.4 GHz
                for _ in range(n):
                    nc.tensor.ldweights(weights=anchor)

            # full k for all 4 heads, l-planar: ktf[d, hh, l, w]; all chunks
            # on the sync queue (the only fast DMA ring - scalar/gpsimd
            # dynamic queues add ~3us of latency)
            ktf = ktp.tile([128, HPC, 16, 512], bf16, name="ktf")
            for e in range(8):
                for hh in range(HPC):
                    nc.sync.dma_start(
                        out=ktf[:, hh, 2 * e : 2 * e + 2, :],
                        in_=ktt[hh, :, 2 * e : 2 * e + 2, :],
                    )

            # stage outputs, layout [d, plane, head, w]
            s0 = s0p.tile([128, 16, HPC, NB], bf16, name="s0")
            sts = [s0]
            for st in range(1, 5):
                sts.append(
                    stp.tile([128, 16 >> st, HPC, NB], bf16, name=f"s{st}")
                )

            # ---- stage 0, e = 0: per-head ramp (ACT starts ~4us earlier) ----
            zp0 = zps.tile([128, HPC, 512], f32, name="zp0", tag="zp")
            for hh in range(HPC):
                # absorber: observe this chunk's DMA semaphore on PE
                nc.tensor.ldweights(weights=ktf[:, hh, 0, 0:128])
                for half in range(2):
                    nc.tensor.matmul(
                        zp0[:, hh, :],
                        lhsT=wd(0, half),
                        rhs=ktf[:, hh, half, :],
                        start=(half == 0),
                        stop=(half == 1),
                    )
                nc.scalar.activation(
                    out=s0[:, 0, hh, :], in_=zp0[:, hh, 0:NB], func=SILU,
                    bias=pe0_sb[:, 0:1], scale=1.0,
                )
                nc.scalar.activation(
                    out=s0[:, 8, hh, :], in_=zp0[:, hh, 1 : NB + 1], func=SILU,
                    bias=pe0_sb[:, 8:9], scale=1.0,
                )

            def stage0_iter(e, split=False):
                zp = zps.tile([128, HPC, 512], f32, name=f"zp{e}", tag="zp")
                if split:
                    # head-major matmuls: a head-pair's activations fire as
                    # soon as that pair's chunks have landed (supply-paced
                    # ramp region)
                    for hh in range(HPC):
                        nc.tensor.ldweights(weights=ktf[:, hh, 2 * e, 0:128])
                        for half in range(2):
                            nc.tensor.matmul(
                                zp[:, hh, :],
                                lhsT=wd(0, half),
                                rhs=ktf[:, hh, 2 * e + half, :],
                                start=(half == 0),
                                stop=(half == 1),
                            )
                else:
                    for hh in range(HPC):
                        nc.tensor.ldweights(weights=ktf[:, hh, 2 * e, 0:128])
                    for half in range(2):
                        for hh in range(HPC):
                            nc.tensor.matmul(
                                zp[:, hh, :],
                                lhsT=wd(0, half),
                                rhs=ktf[:, hh, 2 * e + half, :],
                                start=(half == 0),
                                stop=(half == 1),
                            )
                groups = [(0, 2), (2, 4)] if split else [(0, 4)]
                for a, b in groups:
                    nc.scalar.activation(
                        out=s0[:, e, a:b, :], in_=zp[:, a:b, 0:NB], func=SILU,
                        bias=pe0_sb[:, e : e + 1], scale=1.0,
                    )
                    nc.scalar.activation(
                        out=s0[:, e + 8, a:b, :], in_=zp[:, a:b, 1 : NB + 1],
                        func=SILU,
                        bias=pe0_sb[:, e + 8 : e + 9], scale=1.0,
                    )
                filler(s0[:, e - 1, 0, 0:128], 7)
                filler(s0[:, e + 7, 0, 0:128], 7)

            def merge_iter(st, p, prev_anchor=None, split=False):
                prev, cur = sts[st - 1], sts[st]
                ps = zps.tile([128, HPC, 512], f32, name=f"ps{st}_{p}", tag="zp")
                for half in range(2):
                    for hh in range(HPC):
                        nc.tensor.matmul(
                            ps[:, hh, 0:NB],
                            lhsT=wd(st, half),
                            rhs=prev[:, 2 * p + half, hh, :],
                            start=(half == 0),
                            stop=(half == 1),
                        )
                if split:
                    # head-pair halves so downstream per-head work starts
                    # ~1us earlier (used for the last stage-3 plane)
                    nc.scalar.activation(
                        out=cur[:, p, 0:2, :], in_=ps[:, 0:2, 0:NB], func=SILU,
                    )
                    nc.scalar.activation(
                        out=cur[:, p, 2:4, :], in_=ps[:, 2:4, 0:NB], func=SILU,
                    )
                else:
                    nc.scalar.activation(
                        out=cur[:, p, :, :], in_=ps[:, :, 0:NB], func=SILU,
                    )
                if prev_anchor is not None:
                    filler(prev_anchor, 4)

            # ---- stage 0 e>=1 (sequential: stage 0 is ACT-bound per
            # iteration even with the PE clock-gated cold, so no stage-1
            # work needs to be woven in) ----
            for e in range(1, 8):
                stage0_iter(e)

            # ---- stages 1..3 ----
            anchor = s0[:, 15, 0, 0:128]
            for st in range(1, 4):
                cur = sts[st]
                for p in range(16 >> st):
                    merge_iter(st, p, prev_anchor=anchor)
                    anchor = cur[:, p, 0, 0:128]

            # ---- stage 4 + w_stop + output, pipelined per head ----
            s3, s4 = sts[3], sts[4]
            ps4 = []
            for hh in range(HPC):
                if hh >= 2:
                    # slot WAR: ps4[hh] reuses ps4[hh-2]'s slot; its reader
                    # (act4 hh-2) must be emitted before the new allocation
                    nc.scalar.activation(
                        out=s4[:, 0, hh - 2, :], in_=ps4[hh - 2][:, 0:NB],
                        func=SILU,
                    )
                p4 = zps.tile([128, 512], f32, name=f"ps4_{hh}", tag="zp")
                ps4.append(p4)
                for half in range(2):
                    nc.tensor.matmul(
                        p4[:, 0:NB],
                        lhsT=wd(4, half),
                        rhs=s3[:, half, hh, :],
                        start=(half == 0),
                        stop=(half == 1),
                    )
            for hh in range(2, HPC):
                nc.scalar.activation(
                    out=s4[:, 0, hh, :], in_=ps4[hh][:, 0:NB], func=SILU,
                )

            for hh in range(HPC):
                ps2 = zps.tile([128, 4, 128], f32, name=f"ps2_{hh}", tag="zp")
                for q, (w0, wq) in enumerate(QRANGES):
                    nc.tensor.matmul(
                        ps2[:wq, q, :],
                        lhsT=s4[:, 0, hh, w0 : w0 + wq],
                        rhs=wst_sb,
                        start=True,
                        stop=True,
                    )
                outsb = outp.tile([128, 4, 128], f32, name=f"outsb{hh}", tag="ob")
                # split the PSUM->SBUF drain across the two engines that can
                # read PSUM: ScalarE is idle after its last activation, so
                # heads 1/3 copy there (Copy is in every ACT table set)
                # while heads 0/2 use the DVE - halves the serial copy chain
                if hh % 2 == 0:
                    nc.vector.tensor_copy(out=outsb, in_=ps2)
                else:
                    nc.scalar.copy(out=outsb, in_=ps2)
                filler(s4[:, 0, hh, 0:128], 3)
                nc.sync.dma_start(out=oqs[hh][:], in_=outsb)

    if not nc.is_finalized():
        nc.finalize()
    return nc


def _prep_host_inputs(k, pe, w_down, w_stop):
    import ml_dtypes

    bf16 = ml_dtypes.bfloat16
    k = np.asarray(k, dtype=np.float32)
    pe = np.asarray(pe, dtype=np.float32)
    w_down = np.asarray(w_down, dtype=np.float32)
    w_stop = np.asarray(w_stop, dtype=np.float32)

    # ktt[head, d, l, w] = k[head, 16w + l, d], cast bf16 (RNE): device DMA
    # is then a plain contiguous copy per (plane, head) chunk
    ktt = np.ascontiguousarray(
        k.reshape(BH, 512, 16, D).transpose(0, 3, 2, 1)
    ).astype(bf16)
    # wdt[kk, i, h, o] = w_down[i][o, 128h + kk]
    wdt = np.ascontiguousarray(
        w_down.reshape(5, 128, 2, 128).transpose(3, 0, 2, 1)
    ).astype(bf16)
    # pe0[o, j] = sum_i w_down[0][o, i] * concat(pe[2j], pe[2j+1])[i]
    pe_pairs = pe.reshape(16, 256).astype(np.float64)
    pe0 = (w_down[0].astype(np.float64) @ pe_pairs.T).astype(np.float32)
    wst = np.ascontiguousarray(w_stop.T).astype(bf16)
    return ktt, wdt, pe0, wst


def run(k, pe, w_down, w_stop, trace=False, trace_kwargs=None):
    from concourse.bass_utils import run_bass_kernel_spmd

    ktt, wdt, pe0, wst = _prep_host_inputs(k, pe, w_down, w_stop)

    if "nc" not in _BASS_CACHE:
        _BASS_CACHE["nc"] = _build_bass()
    nc = _BASS_CACHE["nc"]

    in_maps = [
        {
            "ktt": np.ascontiguousarray(ktt[HPC * c : HPC * (c + 1)]),
            "wdt": wdt,
            "pe0": pe0,
            "wst": wst,
        }
        for c in range(NCORES)
    ]
    res = run_bass_kernel_spmd(
        nc, in_maps, core_ids=list(range(NCORES)), trace=trace,
        **(trace_kwargs or {}),
    )
    out = np.empty((BH, NW, D), dtype=np.float32)
    for c in range(NCORES):
        r = res.results[c]
        for hh in range(HPC):
            row = HPC * c + hh
            out[row, 0, :] = 0.0
            # ob[p, q, o] -> out rows 1 + 128q + p  (row 512 doesn't exist)
            ob = r[f"o{hh}"].transpose(1, 0, 2).reshape(512, D)
            out[row, 1:, :] = ob[:NB]
    out = out.reshape(B, H, NW, D)
    return out, res


def kernel(k, pe, w_down, w_stop):
    out, _ = run(k, pe, w_down, w_stop, trace=False)
    return out
